# revision 25
# baseline (speedup 1.0000x reference)
"""AttentionBlock (GroupNorm + single-head full attention + residual) on 8
Trainium2 NeuronCores.

Sharding: data-parallel over batch (4) x sequence-parallel over query
tokens (2 halves of h*w=4096). Each core gets its batch slice with the
token axis ROTATED by the host so that its 2048 queries are always
columns 0:NQ (attention is permutation-invariant over keys, GroupNorm
over positions), so a single xb input serves stats, K, V and the query
slice. No collectives; the host scatters inputs and gathers outputs.

Per-core pipeline (channels on partitions; projections in bf16,
attention in fp8e4 DoubleRow = 2 rows/PE-cell, c=256 contraction in one
128-partition matmul):
 - x arrives as bf16 (host-cast: halves the upload and SBUF; Q/K/V are
   fp8-quantized downstream anyway) over the SP HWDGE ring in 1024-col
   chunks; weights/consts ride the ACT HWDGE ring. No SWDGE DMAs.
 - GroupNorm stats per chunk as DMAs land (DVE sum, ACT Square+accum);
   group reduce + broadcast via tiny indicator matmuls. Normalization
   is FOLDED INTO THE WEIGHTS (W' = W*a per in-channel, bias' =
   W@b + bias) so all matmuls consume RAW bf16 x.
 - W2 = out_w @ Wv and out_b' = out_b + out_w @ bv are folded ON THE
   HOST; Wq|Wk and W2 arrive pre-transposed (wqkT_in, w2T_in).
 - Q, K are written by the projection bias-copies directly as fp8e4 in
   DoubleRow layout [128, 2(c-half), n]; V2^T likewise as fp8 [128,
   32(k-tile), 256].
 - Attention processes 512-query chunks in PAIRS that share every K/V
   stationary load (ldweights dominate DR matmul cost on HW): per
   k-tile one ldw + two S^T matmuls (one per chunk) into the two banks
   of a [128,2,512] PSUM tile; ONE exp per pair-tile (ACT, scale 1/16,
   bias -3; every 3rd via DVE Schraudolph + Pool clamp) emits P^T fp8.
   PV trails and shares V2^T stationaries the same way, accumulating
   O'^T[c,q] per chunk. The softmax denominator l is a separate pass of
   chained DR matmuls with one ones-stationary load over the 32 live
   P^T tiles. 1/l is PE-broadcast to 128 partitions and applied with
   DVE mult (PSUM-capable) + Pool add into the residual y.
 - PSUM: proj uses a 3x[128,1024] ring (6 banks); attention re-slices
   into 2x[128,2,512] S^T pair tiles + 4 O'^T banks; pol/rlb reuse
   vacated S^T slots. Matmul outputs may not cross a 2KB PSUM bank
   (512 f32), so every matmul emits <=512 output columns.

Toolchain notes: walrus accepts one sync-wait per instruction
(SplitWaitTileContext splits the rest onto nops); non-rounding
producers may not feed f32r matmuls (use bf16 instead); gpsimd must
not touch PSUM on HW; SWDGE (gpsimd) DMAs inside a For_i loop break
walrus codegen ("ISA wrong length") - keep all DMAs on HWDGE rings;
fp8 DR needs 3D APs [Ki, 2, dim] with middle step % 16 == 0; PSUM
pools are time-sliced via nested ExitStacks. hw_loop=True wraps the
body in tc.For_i for NEFF-size-independent timing (see test.py).
"""

import numpy as np

B, C, HW = 4, 256, 4096
import math as _math
SCH_A = 8.0 * _math.log2(_math.e) / 16.0
SCH_B = 8.0 * (7.0 - 3.0 * _math.log2(_math.e))
NQ = HW // 2
G = 8
CPG = C // G  # channels per group
EPS = 1e-5
N_CORES = 8
USE_FP8_PV = False
EXP_BIAS = -3.0

_CACHE = {}


def _bf16():
    import ml_dtypes

    return ml_dtypes.bfloat16


_CTX_CACHE = {}


def _get_tc_cls():
    if "cls" in _CTX_CACHE:
        return _CTX_CACHE["cls"]
    import bass_rust
    import concourse.mybir as mybir
    import concourse.tile as tile
    from concourse.vector_clock import ScopedClock

    MAXW = 1

    class SplitWaitTileContext(tile.TileContext):
        """Workaround: this toolchain's walrus accepts at most one sync-wait
        per instruction; split excess waits onto same-engine InstNoOps.
        per_sem_clear avoids the EVENT_SEMAPHORE_RANGE_CLEAR InstISA that
        walrus codegen rejects in modules with control flow (For_i)."""

        per_sem_clear = False

        def _split_excess_waits(self, inst):
            si = inst.sync_info
            if si is None:
                return []
            waits = list(si.on_wait)
            if len(waits) <= MAXW:
                return []
            extra, keep = waits[:-MAXW], waits[-MAXW:]
            nops = [
                mybir.InstNoOp(
                    name=f"I-{self.nc.next_id()}",
                    sync_info=mybir.SyncInfo(on_wait=[w], on_update=[]),
                    bass_nofuse=True,
                    engine=inst.engine,
                )
                for w in extra
            ]
            inst.sync_info = mybir.SyncInfo(on_wait=keep, on_update=list(si.on_update))
            return nops

        def _commit_and_lower(self, inst, original_block, old_bb_map, bb_to_exit_bb):
            for nop in self._split_excess_waits(inst):
                self._commit_instruction(nop, lazy_reg_writes=False)
            return super()._commit_and_lower(
                inst, original_block, old_bb_map, bb_to_exit_bb
            )

        def _drain_and_barrier(self, tick_clock, wait_clock):
            drain_inst = self.nc.sync.drain()
            wait_clock.add_sem_waits(
                drain_inst.ins, ScopedClock({None: tick_clock.global_clock})
            )
            si = drain_inst.ins.sync_info
            waits = list(si.on_wait) if si is not None else []
            if len(waits) > MAXW:
                updates = list(si.on_update) if si is not None else []
                drain_inst.ins.sync_info = bass_rust.SyncInfo(
                    on_wait=waits[:MAXW], on_update=[]
                )
                rest = waits[MAXW:]
                for i, w in enumerate(rest):
                    extra = self.nc.sync.drain()
                    extra.ins.sync_info = bass_rust.SyncInfo(
                        on_wait=[w], on_update=updates if i == len(rest) - 1 else []
                    )
            self.nc.all_engine_barrier()
            assert self.sems is not None
            popped = self.nc._tile_sem_poison_stack.pop()
            assert popped is self._sem_poison
            sems = list(self.sems.allocated().values())
            if self.per_sem_clear:
                nums = sorted(
                    s.num if hasattr(s, "num") else s for s in sems
                )
                for n in nums:
                    self.nc.gpsimd.dma_reset(range(n, n + 1))
                    self.nc.gpsimd.sem_clear(range(n, n + 1))
                self.nc._state.prepend_free_semaphores(nums)
                for ps in self.nc._tile_sem_poison_stack:
                    ps.update(nums)
            else:
                self.nc.clear_and_free_semaphores(sems)
            self.nc.all_engine_barrier()

    _CTX_CACHE["cls"] = SplitWaitTileContext
    return SplitWaitTileContext


def _build_nc(loop_reps=1, debug=False, hw_loop=False):
    import concourse.bass as bass
    import concourse.mybir as mybir
    import concourse.tile as tile

    F32 = mybir.dt.float32
    FR = mybir.dt.float32r
    F8 = mybir.dt.float8e4
    U8 = mybir.dt.uint8
    BF = mybir.dt.bfloat16
    AF = mybir.ActivationFunctionType
    ALU = mybir.AluOpType

    SplitWaitTileContext = _get_tc_cls()

    nc = bass.Bass()
    xb = nc.dram_tensor("xb", [C, HW], BF, kind="ExternalInput")
    qkv_b = nc.dram_tensor("qkv_b", [3 * C], F32, kind="ExternalInput")
    out_b = nc.dram_tensor("out_b", [C], F32, kind="ExternalInput")
    gn_gamma = nc.dram_tensor("gn_gamma", [C], F32, kind="ExternalInput")
    gn_beta = nc.dram_tensor("gn_beta", [C], F32, kind="ExternalInput")
    gind_in = nc.dram_tensor("gind_in", [128, 16], F32, kind="ExternalInput")
    hind_in = nc.dram_tensor("hind_in", [8, 128 * 2], F32, kind="ExternalInput")
    wqkT_in = nc.dram_tensor("wqkT_in", [C, 512], F32, kind="ExternalInput")
    w2T_in = nc.dram_tensor("w2T_in", [C, C], F32, kind="ExternalInput")
    y = nc.dram_tensor("y", [C, NQ], F32, kind="ExternalOutput")
    if debug:
        d_xn = nc.dram_tensor("d_xn", [C, HW], F32, kind="ExternalOutput")
        d_q = nc.dram_tensor("d_q", [C, NQ], F32, kind="ExternalOutput")
        d_k = nc.dram_tensor("d_k", [C, HW], F32, kind="ExternalOutput")
        d_v2t = nc.dram_tensor("d_v2t", [HW, 272], F32, kind="ExternalOutput")
        d_po = nc.dram_tensor("d_po", [128, 272], F32, kind="ExternalOutput")
        d_ab = nc.dram_tensor("d_ab", [C, 2], F32, kind="ExternalOutput")

    with SplitWaitTileContext(nc) as tc:
        import contextlib

        ctx = contextlib.ExitStack()
        with ctx:
            singles = ctx.enter_context(tc.tile_pool(name="singles", bufs=1))
            xpool = ctx.enter_context(tc.tile_pool(name="xpool", bufs=2))
            qpool = ctx.enter_context(tc.tile_pool(name="qpool", bufs=2))
            kpool = ctx.enter_context(tc.tile_pool(name="kpool", bufs=2))
            vpool = ctx.enter_context(tc.tile_pool(name="vpool", bufs=2))
            ypool = ctx.enter_context(tc.tile_pool(name="ypool", bufs=2))
            wpool = ctx.enter_context(tc.tile_pool(name="wpool", bufs=1))
            wnat = ctx.enter_context(tc.tile_pool(name="wnat", bufs=1))
            ppool = ctx.enter_context(tc.tile_pool(name="ppool", bufs=34))
            opool = ctx.enter_context(tc.tile_pool(name="opool", bufs=3))
            small = ctx.enter_context(tc.tile_pool(name="small", bufs=4))
            stat = ctx.enter_context(tc.tile_pool(name="stat", bufs=2))
            scpool = ctx.enter_context(tc.tile_pool(name="scpool", bufs=3))

            def setup():
                # constants: allocate tiles; DMAs deferred so weight/x loads
                # hit the serial DMA-issue paths first.
                eps_sb = singles.tile([8, 1], F32, tag="eps")
                nc.vector.memset(eps_sb, EPS)
                ebias_sb = singles.tile([128, 1], F32, tag="ebias")
                nc.vector.memset(ebias_sb, EXP_BIAS)
                gam_sb = singles.tile([128, 2], F32, tag="gam")
                bet_sb = singles.tile([128, 2], F32, tag="bet")
                qb_sb = singles.tile([128, 6], F32, tag="qb")
                ob_sb = singles.tile([128, 2], F32, tag="ob")
                gi_sb = singles.tile([128, 16], F32, tag="gi")
                hi_sb = singles.tile([8, 128 * 2], F32, tag="hi")

                def load_consts():
                    # consts + weights ride the ACT HWDGE ring; x has the SP
                    # ring to itself.
                    nc.scalar.dma_start(out=qb_sb, in_=qkv_b.rearrange("(m p) -> p m", p=128))
                    nc.scalar.dma_start(out=gam_sb, in_=gn_gamma.rearrange("(t p) -> p t", p=128))
                    nc.scalar.dma_start(out=bet_sb, in_=gn_beta.rearrange("(t p) -> p t", p=128))
                    nc.scalar.dma_start(out=ob_sb, in_=out_b.rearrange("(t p) -> p t", p=128))
                    nc.scalar.dma_start(out=gi_sb, in_=gind_in[:, :])
                    nc.scalar.dma_start(out=hi_sb, in_=hind_in[:, :])

                g_sb = [gam_sb[:, t : t + 1] for t in range(2)]
                be_sb = [bet_sb[:, t : t + 1] for t in range(2)]
                gind = [gi_sb[:, 0:8], gi_sb[:, 8:16]]
                hind = [hi_sb[:, 0:128], hi_sb[:, 128:256]]
                return (g_sb, be_sb, qb_sb, ob_sb, eps_sb, ebias_sb, gind, hind, load_consts)

            def body(rep, consts):
                (g_sb, be_sb, qb_sb, ob_sb, eps_sb, ebias_sb, gind, hind, load_consts) = consts
                # PSUM is time-sliced: proj phase gets a 2x[128,1024] ring
                # (4 banks); attention re-uses the space for S^T pair tiles
                # plus the 4 po accumulators.
                pctx = contextlib.ExitStack()
                psmm = pctx.enter_context(
                    tc.tile_pool(name="psproj", bufs=3, space="PSUM")
                )
                # x on the SP HWDGE ring as plain f32 (f32r is bit-identical;
                # matmuls read .bitcast(FR)). 1024-col chunks so per-chunk
                # stats start as soon as each lands.
                x_sb = [
                    xpool.tile([128, HW], BF, tag="xv", name=f"x{t}")
                    for t in range(2)
                ]
                for c4 in range(4):
                    for t in range(2):
                        nc.sync.dma_start(
                            out=x_sb[t][:, c4 * 1024 : (c4 + 1) * 1024],
                            in_=xb[t * 128 : (t + 1) * 128, c4 * 1024 : (c4 + 1) * 1024],
                        )
                # host pre-rotates xb per core so the query half is always
                # columns 0:NQ (attention is permutation-invariant over keys)
                xq_sb = [x_sb[t][:, 0:NQ] for t in range(2)]

                # weights arrive pre-transposed (and out_w pre-folded into
                # W2 = out_w @ Wv on the host) on the ACT HWDGE ring
                wT = []  # (Wq|Wk)^T tiles [c_in 128, 512] f32
                for t in range(2):
                    wT.append(wpool.tile([128, 512], F32, tag=f"wT{t}", name=f"wTn{t}"))
                w2t = []  # W2^T tiles [c_in 128, 256] f32
                for t in range(2):
                    w2t.append(wpool.tile([128, 256], F32, tag=f"w2t{t}", name=f"w2t{t}"))
                for t in range(2):
                    nc.scalar.dma_start(out=wT[t], in_=wqkT_in[t * 128 : (t + 1) * 128, :])
                for t in range(2):
                    nc.scalar.dma_start(out=w2t[t], in_=w2T_in[t * 128 : (t + 1) * 128, :])
                load_consts()
                ob_eff = ob_sb  # host already folded out_w @ bv into out_b

                # ---------- GroupNorm stats ----------
                # per-channel raw sums: DVE does sum(x) while ACT does
                # sum(x^2) via Square+accum_out (parallel engines).
                st2 = []
                for t in range(2):
                    s1m = stat.tile([128, 4], F32, tag=f"s1m{t}", name=f"s1m{t}")
                    s2m = stat.tile([128, 4], F32, tag=f"s2m{t}", name=f"s2m{t}")
                    for cck in range(4):
                        sl = slice(cck * 1024, (cck + 1) * 1024)
                        nc.vector.reduce_sum(
                            out=s1m[:, cck : cck + 1],
                            in_=x_sb[t][:, sl],
                            axis=mybir.AxisListType.X,
                        )
                        sq = scpool.tile([128, 1024], BF, tag="sc", name=f"sq{t}{cck}")
                        nc.scalar.activation(
                            out=sq, in_=x_sb[t][:, sl],
                            func=AF.Square, accum_out=s2m[:, cck : cck + 1],
                        )
                    s2t = stat.tile([128, 2], F32, tag=f"st2{t}")
                    nc.vector.reduce_sum(out=s2t[:, 0:1], in_=s1m, axis=mybir.AxisListType.X)
                    nc.vector.reduce_sum(out=s2t[:, 1:2], in_=s2m, axis=mybir.AxisListType.X)
                    st2.append(s2t)
                psg = psmm.tile([8, 2], F32, tag="mm")
                nc.tensor.matmul(psg, gind[0], st2[0], start=True, stop=False)
                nc.tensor.matmul(psg, gind[1], st2[1], start=False, stop=True)
                gstat = stat.tile([8, 2], F32, tag="gstat")  # [mean_g, E[x^2]_g]
                nc.vector.tensor_scalar_mul(gstat, psg, 1.0 / (CPG * HW))
                var_g = stat.tile([8, 1], F32, tag="varg")
                nc.vector.tensor_mul(var_g, gstat[:, 0:1], gstat[:, 0:1])
                nc.vector.tensor_sub(var_g, gstat[:, 1:2], var_g)
                std_g = stat.tile([8, 1], F32, tag="stdg")
                nc.scalar.activation(out=std_g, in_=var_g, func=AF.Sqrt, bias=eps_sb, scale=1.0)
                # preload the exp table set off the attention critical path
                warm = stat.tile([8, 1], F32, tag="warm")
                nc.scalar.activation(out=warm, in_=eps_sb, func=AF.Exp, scale=1.0)
                rm = stat.tile([8, 2], F32, tag="rm")  # [rstd_g, mean_g]
                nc.vector.reciprocal(rm[:, 0:1], std_g)
                nc.vector.tensor_copy(rm[:, 1:2], gstat[:, 0:1])
                # broadcast to channels: [rstd_c, mean_c] = H_t.T @ rm
                ab = []
                for t in range(2):
                    psb = psmm.tile([128, 2], F32, tag="mm")
                    nc.tensor.matmul(psb, hind[t], rm, start=True, stop=True)
                    abt = stat.tile([128, 2], F32, tag=f"ab{t}")  # [a_c, b_c]
                    nc.vector.tensor_mul(abt[:, 0:1], psb[:, 0:1], g_sb[t])
                    nc.vector.tensor_mul(abt[:, 1:2], psb[:, 1:2], abt[:, 0:1])
                    nc.vector.tensor_sub(abt[:, 1:2], be_sb[t], abt[:, 1:2])
                    ab.append(abt)

                # ---------- fold GN into weights: no x_norm pass ----------
                # K/Q/V2 consume RAW x; W' = W * a (per c_in), biases get W@b.
                # Bias matmuls (plain fp32, N=1-2) use the UNSCALED weights;
                # the in-place scales below are WAR-ordered after them.
                wTs = []
                w2ts = []
                for t in range(2):
                    wt2 = wpool.tile([128, 512], BF, tag=f"wTs{t}", name=f"wTs{t}")
                    nc.vector.tensor_scalar_mul(wt2, wT[t], ab[t][:, 0:1])
                    wTs.append(wt2)
                    w22 = wpool.tile([128, 256], BF, tag=f"w2ts{t}", name=f"w2ts{t}")
                    nc.vector.tensor_scalar_mul(w22, w2t[t], ab[t][:, 0:1])
                    w2ts.append(w22)
                ps_qb = psmm.tile([128, 4], F32, tag="mm", name="ps_qb")
                for m in range(4):
                    nc.tensor.matmul(
                        ps_qb[:, m : m + 1],
                        wT[0][:, m * 128 : (m + 1) * 128],
                        ab[0][:, 1:2],
                        start=True, stop=False,
                    )
                    nc.tensor.matmul(
                        ps_qb[:, m : m + 1],
                        wT[1][:, m * 128 : (m + 1) * 128],
                        ab[1][:, 1:2],
                        start=False, stop=True,
                    )
                qb_eff = stat.tile([128, 4], F32, tag="qbeff")
                nc.vector.tensor_add(qb_eff, ps_qb, qb_sb[:, 0:4])
                ps_ob2 = psmm.tile([128, 2], F32, tag="mm", name="ps_ob2")
                for m2 in range(2):
                    nc.tensor.matmul(
                        ps_ob2[:, m2 : m2 + 1],
                        w2t[0][:, m2 * 128 : (m2 + 1) * 128],
                        ab[0][:, 1:2],
                        start=True, stop=False,
                    )
                    nc.tensor.matmul(
                        ps_ob2[:, m2 : m2 + 1],
                        w2t[1][:, m2 * 128 : (m2 + 1) * 128],
                        ab[1][:, 1:2],
                        start=False, stop=True,
                    )
                ob_f = stat.tile([128, 2], F32, tag="obf")
                nc.vector.tensor_add(ob_f, ps_ob2, ob_eff)

                # residual prep from raw xq bits
                y_sb = []
                for t in range(2):
                    yt = ypool.tile([128, NQ], F32, tag="y", name=f"y{t}")
                    nc.vector.tensor_scalar_add(
                        yt, xq_sb[t][:, :], ob_f[:, t : t + 1]
                    )
                    y_sb.append(yt)
                xn = x_sb
                xqn = xq_sb

                # ---------- qkv projections ----------
                # Q and K are written as fp8e4 in DoubleRow layout
                # [128, 2, n] (c-halves packed in the middle dim) so the
                # scores matmul runs one fp8 DR matmul per (kt, qc).
                q8 = qpool.tile([128, 2, NQ], F8, tag="q", name="q8")
                k8 = kpool.tile([128, 2, HW], F8, tag="k", name="k8")
                nch = 0
                for m in (2, 3, 0, 1):
                    dst = q8 if m < 2 else k8
                    src = xqn if m < 2 else xn
                    nj = (NQ if m < 2 else HW) // 1024
                    for j in range(nj):
                        ps = psmm.tile([128, 1024], F32, tag="mm")
                        for i in range(2):
                            sl = slice((2 * j + i) * 512, (2 * j + i + 1) * 512)
                            nc.tensor.matmul(
                                ps[:, i * 512 : (i + 1) * 512],
                                wTs[0][:, m * 128 : (m + 1) * 128],
                                src[0][:, sl],
                                start=True,
                                stop=False,
                            )
                            nc.tensor.matmul(
                                ps[:, i * 512 : (i + 1) * 512],
                                wTs[1][:, m * 128 : (m + 1) * 128],
                                src[1][:, sl],
                                start=False,
                                stop=True,
                            )
                        dslice = dst[:, m % 2, j * 1024 : (j + 1) * 1024]
                        if nch % 2 == 0:
                            nc.vector.tensor_scalar_add(dslice, ps, qb_eff[:, m : m + 1])
                        else:
                            nc.scalar.activation(
                                out=dslice, in_=ps, func=AF.Identity,
                                bias=qb_eff[:, m : m + 1], scale=1.0,
                            )
                        nch += 1

                # ---------- V2^T = xn^T @ W2^T, fp8 [k-tile, c] ----------
                # v28[p, nt, c] = V2^T[nt*128+p, c]; PV runs transposed
                # (stationary = V2T k-pair slices, moving = P^T) so the
                # stationary is reused and fp8 DoubleRow halves the stream.
                v28 = vpool.tile([128, 32, 256], F8, tag="v2", name="v28")
                for ntq in range(8):
                    ps = psmm.tile([128, 1024], F32, tag="mm")
                    for i in range(4):
                        nt = 4 * ntq + i
                        nc.tensor.matmul(
                            ps[:, i * 256 : (i + 1) * 256],
                            xn[0][:, nt * 128 : (nt + 1) * 128], w2ts[0],
                            start=True, stop=False,
                        )
                        nc.tensor.matmul(
                            ps[:, i * 256 : (i + 1) * 256],
                            xn[1][:, nt * 128 : (nt + 1) * 128], w2ts[1],
                            start=False, stop=True,
                        )
                    dst = v28[:, 4 * ntq : 4 * ntq + 4, :]
                    if ntq % 2 == 0:
                        nc.vector.tensor_copy(dst, ps)
                    else:
                        nc.scalar.copy(dst, ps)
                # fp8 DR stationary needs middle-dim step % 16 == 0 and
                # memset can't write fp8: build the ones stationary as
                # [128, 2, 16] via an ACT copy from an f32 ones tile.
                ones_f = singles.tile([128, 32], F32, tag="onesf")
                nc.vector.memset(ones_f, 1.0)
                ones8 = singles.tile([128, 2, 16], F8, tag="ones8")
                nc.scalar.copy(ones8, ones_f)
                ones1f = singles.tile([1, 128], F32, tag="ones1f")
                nc.vector.memset(ones1f, 1.0)
                ones1 = singles.tile([1, 128], FR, tag="ones1")
                nc.vector.tensor_copy(ones1, ones1f)

                if debug:
                    for t in range(2):
                        nc.sync.dma_start(
                            out=d_xn[t * 128 : (t + 1) * 128, :],
                            in_=xn[t][:, :],
                        )
                        nc.sync.dma_start(
                            out=d_ab[t * 128 : (t + 1) * 128, :], in_=ab[t]
                        )

                # ---------- attention ----------
                # Per 512-query chunk (4 chunks): the two S^T k-tiles of a
                # jp-pair land in the two banks of ONE [128,2,512] PSUM tile
                # (ring 3 = 6 banks; matmul outputs may not cross a bank, but
                # exp reads may), so ONE exp per pair [128,2,512]->pT8
                # halves the exp instruction count. PV trails by 1 pair and
                # accumulates O'^T [128,512] x2 c-halves (2 banks). All 16
                # pT8 tiles stay live; the softmax denominator l is a
                # SEPARATE pass of 16 back-to-back DR matmuls with one ones
                # stationary load, into a ring slot vacated by S^T. 1/l is
                # PE-broadcast and applied on DVE (ch0) + Pool (ch1).
                pctx.close()
                actx = contextlib.ExitStack()
                psattn = actx.enter_context(
                    tc.tile_pool(name="psattn", bufs=2, space="PSUM")
                )
                pspo = actx.enter_context(
                    tc.tile_pool(name="pspo", bufs=4, space="PSUM")
                )
                NQB = 512
                for qg in range(NQ // (2 * NQB)):
                    # two query chunks (a=0, b=1) share every K/V stationary
                    # load: ldw(kt) then S-mm for both chunks back to back.
                    qsl = [
                        slice((2 * qg + h) * NQB, (2 * qg + h + 1) * NQB)
                        for h in range(2)
                    ]
                    poT = [
                        [
                            pspo.tile([128, NQB], F32, tag="o", name=f"poT{h}{ch}")
                            for ch in range(2)
                        ]
                        for h in range(2)
                    ]

                    def pv_stage(jp, pT8h):
                        for ch in range(2):
                            for h in range(2):
                                nc.tensor.matmul(
                                    poT[h][ch],
                                    v28[:, 2 * jp : 2 * jp + 2, ch * 128 : (ch + 1) * 128],
                                    pT8h[h],
                                    start=(jp == 0), stop=(jp == 15),
                                    perf_mode=mybir.MatmulPerfMode.DoubleRow,
                                    skip_group_check=True,
                                )

                    pv_pend = []
                    pT8s = [[], []]
                    for jp in range(16):
                        pT8h = [
                            ppool.tile([128, 2, NQB], F8, tag="p", name=f"pT8_{jp}_{h}")
                            for h in range(2)
                        ]
                        for h in range(2):
                            pT8s[h].append(pT8h[h])
                        psph = [
                            psattn.tile([128, 2, NQB], F32, tag="pair", name=f"psp_{jp}_{h}")
                            for h in range(2)
                        ]
                        for i in range(2):
                            kt = 2 * jp + i
                            for h in range(2):
                                nc.tensor.matmul(
                                    psph[h][:, i, :],
                                    k8[:, :, kt * 128 : (kt + 1) * 128],
                                    q8[:, :, qsl[h]],
                                    start=True, stop=True,
                                    perf_mode=mybir.MatmulPerfMode.DoubleRow,
                                )
                        for h in range(2):
                            if (jp + h) % 3 == 2:
                                # Schraudolph-to-e4m3: P bits = clamp(S*A+B)
                                # on the otherwise-idle DVE+Pool.
                                st = scpool.tile(
                                    [128, 2 * NQB], F32, tag="sc", name="sch"
                                )
                                nc.vector.tensor_scalar(
                                    out=st, in0=psph[h],
                                    scalar1=SCH_A, scalar2=SCH_B,
                                    op0=ALU.mult, op1=ALU.add,
                                )
                                nc.gpsimd.tensor_scalar_max(
                                    pT8h[h].bitcast(U8), st, 0.0
                                )
                            else:
                                nc.scalar.activation(
                                    out=pT8h[h], in_=psph[h], func=AF.Exp,
                                    scale=1.0 / 16.0, bias=ebias_sb,
                                )
                        pv_pend.append((jp, pT8h))
                        if len(pv_pend) > 1:
                            pv_stage(*pv_pend.pop(0))
                    for it in pv_pend:
                        pv_stage(*it)
                    # denominator pass: one stationary load, 32 chained mms
                    pol = [
                        psattn.tile([2, NQB], F32, tag="pair", name=f"pol{h}")
                        for h in range(2)
                    ]
                    for jp in range(16):
                        for h in range(2):
                            nc.tensor.matmul(
                                pol[h],
                                ones8[:, :, 0:2],
                                pT8s[h][jp],
                                start=(jp == 0), stop=(jp == 15),
                                perf_mode=mybir.MatmulPerfMode.DoubleRow,
                                skip_group_check=True,
                            )
                    for h in range(2):
                        # rl = 1/l broadcast to all 128 partitions via PE
                        rlv = small.tile([1, NQB], FR, tag="rlv")
                        with nc.allow_low_precision(reason="rl bcast via f32r matmul"):
                            nc.vector.reciprocal(rlv, pol[h][0:1, :])
                        rlb = psattn.tile([128, NQB], F32, tag="pair", name=f"rlb{h}")
                        nc.tensor.matmul(rlb, ones1, rlv, start=True, stop=True)
                        # engines may read only one PSUM operand per instruction
                        rlb_sb = opool.tile([128, NQB], F32, tag="rlbs", name=f"rlb_sb{h}")
                        nc.vector.tensor_copy(rlb_sb, rlb)
                        tmps = []
                        for t in range(2):
                            tmp = opool.tile([128, NQB], F32, tag="tmp", name=f"tmp{h}{t}")
                            nc.vector.tensor_tensor(tmp, poT[h][t], rlb_sb, ALU.mult)
                            tmps.append(tmp)
                        for t in range(2):
                            ys = y_sb[t][:, qsl[h]]
                            nc.gpsimd.tensor_tensor(ys, tmps[t], ys, ALU.add)
                        for t in range(2):
                            nc.sync.dma_start(
                                out=y[t * 128 : (t + 1) * 128, qsl[h]],
                                in_=y_sb[t][:, qsl[h]],
                            )
                actx.close()

            consts = setup()
            if hw_loop and loop_reps > 1:
                with tc.For_i(0, loop_reps) as _i:
                    body(0, consts)
            else:
                for rep in range(loop_reps):
                    body(rep, consts)

    return nc


def _get_runner(loop_reps=1):
    key = ("runner", loop_reps)
    if key not in _CACHE:
        nc = _build_nc(loop_reps)
        _CACHE[key] = nc
    return _CACHE[key]


K_USE_FP8 = USE_FP8_PV


def make_extra_inputs():
    gind = np.zeros((128, 16), dtype=np.float32)
    hind = np.zeros((8, 256), dtype=np.float32)
    for t in range(2):
        for p in range(128):
            g = (t * 128 + p) // CPG
            gind[p, t * 8 + g] = 1.0
            hind[g, t * 128 + p] = 1.0
    return {"gind_in": gind, "hind_in": hind}


def make_weight_inputs(qkv_w, out_w, qkv_b, out_b):
    # host-side static folds: W2 = out_w @ Wv; out_b' = out_b + out_w @ bv
    qkv_w = np.asarray(qkv_w, dtype=np.float32)
    out_w = np.asarray(out_w, dtype=np.float32)
    w2 = out_w @ qkv_w[512:768]
    ob = np.asarray(out_b, dtype=np.float32) + out_w @ np.asarray(
        qkv_b, dtype=np.float32
    )[512:768]
    return {
        "wqkT_in": np.ascontiguousarray(qkv_w[0:512].T),
        "w2T_in": np.ascontiguousarray(w2.T),
        "out_b": ob,
    }


def kernel(x, gn_gamma, gn_beta, qkv_w, qkv_b, out_w, out_b):
    from concourse.bass_utils import run_bass_kernel_spmd

    x = np.asarray(x, dtype=np.float32)
    gn_gamma = np.asarray(gn_gamma, dtype=np.float32)
    gn_beta = np.asarray(gn_beta, dtype=np.float32)
    qkv_w = np.asarray(qkv_w, dtype=np.float32)
    qkv_b = np.asarray(qkv_b, dtype=np.float32)
    out_w = np.asarray(out_w, dtype=np.float32)
    out_b = np.asarray(out_b, dtype=np.float32)

    b, c, h, w = x.shape
    assert (b, c, h * w) == (B, C, HW)
    xf = x.reshape(b, c, HW)

    nc = _get_runner()
    in_maps = []
    for j in range(N_CORES):
        bi, qh = j // 2, j % 2
        if qh == 0:
            xbj = np.ascontiguousarray(xf[bi])
        else:
            xbj = np.concatenate([xf[bi][:, NQ:], xf[bi][:, :NQ]], axis=1)
        xbj = xbj.astype(_bf16())
        in_maps.append(
            {
                "xb": xbj,
                "qkv_b": qkv_b,
                "out_b": out_b,
                "gn_gamma": gn_gamma,
                "gn_beta": gn_beta,
            }
        )
    extras = make_extra_inputs()
    extras.update(make_weight_inputs(qkv_w, out_w, qkv_b, out_b))
    for m in in_maps:
        m.update(extras)
    res = run_bass_kernel_spmd(nc, in_maps, core_ids=list(range(N_CORES)))
    out = np.empty((B, C, HW), dtype=np.float32)
    for j in range(N_CORES):
        bi, qh = j // 2, j % 2
        out[bi][:, qh * NQ : (qh + 1) * NQ] = res.results[j]["y"]
    return out.reshape(b, c, h, w)



# revision 26
# speedup vs baseline: 2.7642x; 2.7642x over previous
"""AttentionBlock (GroupNorm + single-head full attention + residual) on 8
Trainium2 NeuronCores.

Sharding: data-parallel over batch (4) x sequence-parallel over query
tokens (2 halves of h*w=4096). Each core gets its batch slice with the
token axis ROTATED by the host so that its 2048 queries are always
columns 0:NQ (attention is permutation-invariant over keys, GroupNorm
over positions), so a single xb input serves stats, K, V and the query
slice. No collectives; the host scatters inputs and gathers outputs.

Per-core pipeline (channels on partitions; projections in bf16,
attention in fp8e4 DoubleRow = 2 rows/PE-cell, c=256 contraction in one
128-partition matmul):
 - x arrives as bf16 (host-cast: halves the upload and SBUF; Q/K/V are
   fp8-quantized downstream anyway) over the SP HWDGE ring in 1024-col
   chunks; weights/consts ride the ACT HWDGE ring. No SWDGE DMAs.
 - GroupNorm stats per chunk as DMAs land (DVE sum, ACT Square+accum);
   group reduce + broadcast via tiny indicator matmuls. Normalization
   is FOLDED INTO THE WEIGHTS (W' = W*a per in-channel, bias' =
   W@b + bias) so all matmuls consume RAW bf16 x.
 - W2 = out_w @ Wv and out_b' = out_b + out_w @ bv are folded ON THE
   HOST; Wq|Wk and W2 arrive pre-transposed (wqkT_in, w2T_in).
 - Q, K are written by the projection bias-copies directly as fp8e4 in
   DoubleRow layout [128, 2(c-half), n]; V2^T likewise as fp8 [128,
   32(k-tile), 256].
 - Attention processes 512-query chunks in PAIRS that share every K/V
   stationary load (ldweights dominate DR matmul cost on HW): per
   k-tile one ldw + two S^T matmuls (one per chunk) into the two banks
   of a [128,2,512] PSUM tile; ONE exp per pair-tile on ACT (scale
   1/16, bias -3) emits P^T fp8 - all exps on ACT: the DVE Schraudolph
   + Pool-clamp alternative measured 2.5x SLOWER end-to-end on HW.
   PV trails and shares V2^T stationaries the same way, accumulating
   O'^T[c,q] per chunk. The softmax denominator l is a separate pass of
   chained DR matmuls with one ones-stationary load over the 32 live
   P^T tiles. 1/l is PE-broadcast to 128 partitions and applied with
   DVE mult (PSUM-capable) + Pool add into the residual y.
 - PSUM: proj uses a 3x[128,1024] ring (6 banks); attention re-slices
   into 2x[128,2,512] S^T pair tiles + 4 O'^T banks; pol/rlb reuse
   vacated S^T slots. Matmul outputs may not cross a 2KB PSUM bank
   (512 f32), so every matmul emits <=512 output columns.

Toolchain notes: walrus accepts one sync-wait per instruction
(SplitWaitTileContext splits the rest onto nops); non-rounding
producers may not feed f32r matmuls (use bf16 instead); gpsimd must
not touch PSUM on HW; SWDGE (gpsimd) DMAs inside a For_i loop break
walrus codegen ("ISA wrong length") - keep all DMAs on HWDGE rings;
fp8 DR needs 3D APs [Ki, 2, dim] with middle step % 16 == 0; PSUM
pools are time-sliced via nested ExitStacks. hw_loop=True wraps the
body in tc.For_i for NEFF-size-independent timing (see test.py).
"""

import numpy as np

B, C, HW = 4, 256, 4096
import math as _math
SCH_A = 8.0 * _math.log2(_math.e) / 16.0
SCH_B = 8.0 * (7.0 - 3.0 * _math.log2(_math.e))
NQ = HW // 2
G = 8
CPG = C // G  # channels per group
EPS = 1e-5
N_CORES = 8
USE_FP8_PV = False
EXP_BIAS = -3.0

_CACHE = {}


def _bf16():
    import ml_dtypes

    return ml_dtypes.bfloat16


_CTX_CACHE = {}


def _get_tc_cls():
    if "cls" in _CTX_CACHE:
        return _CTX_CACHE["cls"]
    import bass_rust
    import concourse.mybir as mybir
    import concourse.tile as tile
    from concourse.vector_clock import ScopedClock

    MAXW = 1

    class SplitWaitTileContext(tile.TileContext):
        """Workaround: this toolchain's walrus accepts at most one sync-wait
        per instruction; split excess waits onto same-engine InstNoOps.
        per_sem_clear avoids the EVENT_SEMAPHORE_RANGE_CLEAR InstISA that
        walrus codegen rejects in modules with control flow (For_i)."""

        per_sem_clear = False

        def _split_excess_waits(self, inst):
            si = inst.sync_info
            if si is None:
                return []
            waits = list(si.on_wait)
            if len(waits) <= MAXW:
                return []
            extra, keep = waits[:-MAXW], waits[-MAXW:]
            nops = [
                mybir.InstNoOp(
                    name=f"I-{self.nc.next_id()}",
                    sync_info=mybir.SyncInfo(on_wait=[w], on_update=[]),
                    bass_nofuse=True,
                    engine=inst.engine,
                )
                for w in extra
            ]
            inst.sync_info = mybir.SyncInfo(on_wait=keep, on_update=list(si.on_update))
            return nops

        def _commit_and_lower(self, inst, original_block, old_bb_map, bb_to_exit_bb):
            for nop in self._split_excess_waits(inst):
                self._commit_instruction(nop, lazy_reg_writes=False)
            return super()._commit_and_lower(
                inst, original_block, old_bb_map, bb_to_exit_bb
            )

        def _drain_and_barrier(self, tick_clock, wait_clock):
            drain_inst = self.nc.sync.drain()
            wait_clock.add_sem_waits(
                drain_inst.ins, ScopedClock({None: tick_clock.global_clock})
            )
            si = drain_inst.ins.sync_info
            waits = list(si.on_wait) if si is not None else []
            if len(waits) > MAXW:
                updates = list(si.on_update) if si is not None else []
                drain_inst.ins.sync_info = bass_rust.SyncInfo(
                    on_wait=waits[:MAXW], on_update=[]
                )
                rest = waits[MAXW:]
                for i, w in enumerate(rest):
                    extra = self.nc.sync.drain()
                    extra.ins.sync_info = bass_rust.SyncInfo(
                        on_wait=[w], on_update=updates if i == len(rest) - 1 else []
                    )
            self.nc.all_engine_barrier()
            assert self.sems is not None
            popped = self.nc._tile_sem_poison_stack.pop()
            assert popped is self._sem_poison
            sems = list(self.sems.allocated().values())
            if self.per_sem_clear:
                nums = sorted(
                    s.num if hasattr(s, "num") else s for s in sems
                )
                for n in nums:
                    self.nc.gpsimd.dma_reset(range(n, n + 1))
                    self.nc.gpsimd.sem_clear(range(n, n + 1))
                self.nc._state.prepend_free_semaphores(nums)
                for ps in self.nc._tile_sem_poison_stack:
                    ps.update(nums)
            else:
                self.nc.clear_and_free_semaphores(sems)
            self.nc.all_engine_barrier()

    _CTX_CACHE["cls"] = SplitWaitTileContext
    return SplitWaitTileContext


def _build_nc(loop_reps=1, debug=False, hw_loop=False):
    import concourse.bass as bass
    import concourse.mybir as mybir
    import concourse.tile as tile

    F32 = mybir.dt.float32
    FR = mybir.dt.float32r
    F8 = mybir.dt.float8e4
    U8 = mybir.dt.uint8
    BF = mybir.dt.bfloat16
    AF = mybir.ActivationFunctionType
    ALU = mybir.AluOpType

    SplitWaitTileContext = _get_tc_cls()

    nc = bass.Bass()
    xb = nc.dram_tensor("xb", [C, HW], BF, kind="ExternalInput")
    qkv_b = nc.dram_tensor("qkv_b", [3 * C], F32, kind="ExternalInput")
    out_b = nc.dram_tensor("out_b", [C], F32, kind="ExternalInput")
    gn_gamma = nc.dram_tensor("gn_gamma", [C], F32, kind="ExternalInput")
    gn_beta = nc.dram_tensor("gn_beta", [C], F32, kind="ExternalInput")
    gind_in = nc.dram_tensor("gind_in", [128, 16], F32, kind="ExternalInput")
    hind_in = nc.dram_tensor("hind_in", [8, 128 * 2], F32, kind="ExternalInput")
    wqkT_in = nc.dram_tensor("wqkT_in", [C, 512], F32, kind="ExternalInput")
    w2T_in = nc.dram_tensor("w2T_in", [C, C], F32, kind="ExternalInput")
    y = nc.dram_tensor("y", [C, NQ], F32, kind="ExternalOutput")
    if debug:
        d_xn = nc.dram_tensor("d_xn", [C, HW], F32, kind="ExternalOutput")
        d_q = nc.dram_tensor("d_q", [C, NQ], F32, kind="ExternalOutput")
        d_k = nc.dram_tensor("d_k", [C, HW], F32, kind="ExternalOutput")
        d_v2t = nc.dram_tensor("d_v2t", [HW, 272], F32, kind="ExternalOutput")
        d_po = nc.dram_tensor("d_po", [128, 272], F32, kind="ExternalOutput")
        d_ab = nc.dram_tensor("d_ab", [C, 2], F32, kind="ExternalOutput")

    with SplitWaitTileContext(nc) as tc:
        import contextlib

        ctx = contextlib.ExitStack()
        with ctx:
            singles = ctx.enter_context(tc.tile_pool(name="singles", bufs=1))
            xpool = ctx.enter_context(tc.tile_pool(name="xpool", bufs=2))
            qpool = ctx.enter_context(tc.tile_pool(name="qpool", bufs=2))
            kpool = ctx.enter_context(tc.tile_pool(name="kpool", bufs=2))
            vpool = ctx.enter_context(tc.tile_pool(name="vpool", bufs=2))
            ypool = ctx.enter_context(tc.tile_pool(name="ypool", bufs=2))
            wpool = ctx.enter_context(tc.tile_pool(name="wpool", bufs=1))
            wnat = ctx.enter_context(tc.tile_pool(name="wnat", bufs=1))
            ppool = ctx.enter_context(tc.tile_pool(name="ppool", bufs=34))
            opool = ctx.enter_context(tc.tile_pool(name="opool", bufs=3))
            small = ctx.enter_context(tc.tile_pool(name="small", bufs=4))
            stat = ctx.enter_context(tc.tile_pool(name="stat", bufs=2))
            scpool = ctx.enter_context(tc.tile_pool(name="scpool", bufs=3))

            def setup():
                # constants: allocate tiles; DMAs deferred so weight/x loads
                # hit the serial DMA-issue paths first.
                eps_sb = singles.tile([8, 1], F32, tag="eps")
                nc.vector.memset(eps_sb, EPS)
                ebias_sb = singles.tile([128, 1], F32, tag="ebias")
                nc.vector.memset(ebias_sb, EXP_BIAS)
                gam_sb = singles.tile([128, 2], F32, tag="gam")
                bet_sb = singles.tile([128, 2], F32, tag="bet")
                qb_sb = singles.tile([128, 6], F32, tag="qb")
                ob_sb = singles.tile([128, 2], F32, tag="ob")
                gi_sb = singles.tile([128, 16], F32, tag="gi")
                hi_sb = singles.tile([8, 128 * 2], F32, tag="hi")

                def load_consts():
                    # consts + weights ride the ACT HWDGE ring; x has the SP
                    # ring to itself.
                    nc.scalar.dma_start(out=qb_sb, in_=qkv_b.rearrange("(m p) -> p m", p=128))
                    nc.scalar.dma_start(out=gam_sb, in_=gn_gamma.rearrange("(t p) -> p t", p=128))
                    nc.scalar.dma_start(out=bet_sb, in_=gn_beta.rearrange("(t p) -> p t", p=128))
                    nc.scalar.dma_start(out=ob_sb, in_=out_b.rearrange("(t p) -> p t", p=128))
                    nc.scalar.dma_start(out=gi_sb, in_=gind_in[:, :])
                    nc.scalar.dma_start(out=hi_sb, in_=hind_in[:, :])

                g_sb = [gam_sb[:, t : t + 1] for t in range(2)]
                be_sb = [bet_sb[:, t : t + 1] for t in range(2)]
                gind = [gi_sb[:, 0:8], gi_sb[:, 8:16]]
                hind = [hi_sb[:, 0:128], hi_sb[:, 128:256]]
                return (g_sb, be_sb, qb_sb, ob_sb, eps_sb, ebias_sb, gind, hind, load_consts)

            def body(rep, consts):
                (g_sb, be_sb, qb_sb, ob_sb, eps_sb, ebias_sb, gind, hind, load_consts) = consts
                # PSUM is time-sliced: proj phase gets a 2x[128,1024] ring
                # (4 banks); attention re-uses the space for S^T pair tiles
                # plus the 4 po accumulators.
                pctx = contextlib.ExitStack()
                psmm = pctx.enter_context(
                    tc.tile_pool(name="psproj", bufs=3, space="PSUM")
                )
                # x on the SP HWDGE ring as plain f32 (f32r is bit-identical;
                # matmuls read .bitcast(FR)). 1024-col chunks so per-chunk
                # stats start as soon as each lands.
                x_sb = [
                    xpool.tile([128, HW], BF, tag="xv", name=f"x{t}")
                    for t in range(2)
                ]
                for c4 in range(4):
                    for t in range(2):
                        eng = nc.sync if t == 0 else nc.scalar
                        eng.dma_start(
                            out=x_sb[t][:, c4 * 1024 : (c4 + 1) * 1024],
                            in_=xb[t * 128 : (t + 1) * 128, c4 * 1024 : (c4 + 1) * 1024],
                        )
                # host pre-rotates xb per core so the query half is always
                # columns 0:NQ (attention is permutation-invariant over keys)
                xq_sb = [x_sb[t][:, 0:NQ] for t in range(2)]

                # weights arrive pre-transposed (and out_w pre-folded into
                # W2 = out_w @ Wv on the host) on the ACT HWDGE ring
                wT = []  # (Wq|Wk)^T tiles [c_in 128, 512] f32
                for t in range(2):
                    wT.append(wpool.tile([128, 512], F32, tag=f"wT{t}", name=f"wTn{t}"))
                w2t = []  # W2^T tiles [c_in 128, 256] f32
                for t in range(2):
                    w2t.append(wpool.tile([128, 256], F32, tag=f"w2t{t}", name=f"w2t{t}"))
                for t in range(2):
                    nc.scalar.dma_start(out=wT[t], in_=wqkT_in[t * 128 : (t + 1) * 128, :])
                for t in range(2):
                    nc.scalar.dma_start(out=w2t[t], in_=w2T_in[t * 128 : (t + 1) * 128, :])
                load_consts()
                ob_eff = ob_sb  # host already folded out_w @ bv into out_b

                # ---------- GroupNorm stats ----------
                # per-channel raw sums: DVE does sum(x) while ACT does
                # sum(x^2) via Square+accum_out (parallel engines).
                st2 = []
                for t in range(2):
                    s1m = stat.tile([128, 4], F32, tag=f"s1m{t}", name=f"s1m{t}")
                    s2m = stat.tile([128, 4], F32, tag=f"s2m{t}", name=f"s2m{t}")
                    for cck in range(4):
                        sl = slice(cck * 1024, (cck + 1) * 1024)
                        nc.vector.reduce_sum(
                            out=s1m[:, cck : cck + 1],
                            in_=x_sb[t][:, sl],
                            axis=mybir.AxisListType.X,
                        )
                        sq = scpool.tile([128, 1024], BF, tag="sc", name=f"sq{t}{cck}")
                        nc.scalar.activation(
                            out=sq, in_=x_sb[t][:, sl],
                            func=AF.Square, accum_out=s2m[:, cck : cck + 1],
                        )
                    s2t = stat.tile([128, 2], F32, tag=f"st2{t}")
                    nc.vector.reduce_sum(out=s2t[:, 0:1], in_=s1m, axis=mybir.AxisListType.X)
                    nc.vector.reduce_sum(out=s2t[:, 1:2], in_=s2m, axis=mybir.AxisListType.X)
                    st2.append(s2t)
                psg = psmm.tile([8, 2], F32, tag="mm")
                nc.tensor.matmul(psg, gind[0], st2[0], start=True, stop=False)
                nc.tensor.matmul(psg, gind[1], st2[1], start=False, stop=True)
                gstat = stat.tile([8, 2], F32, tag="gstat")  # [mean_g, E[x^2]_g]
                nc.vector.tensor_scalar_mul(gstat, psg, 1.0 / (CPG * HW))
                var_g = stat.tile([8, 1], F32, tag="varg")
                nc.vector.tensor_mul(var_g, gstat[:, 0:1], gstat[:, 0:1])
                nc.vector.tensor_sub(var_g, gstat[:, 1:2], var_g)
                std_g = stat.tile([8, 1], F32, tag="stdg")
                nc.scalar.activation(out=std_g, in_=var_g, func=AF.Sqrt, bias=eps_sb, scale=1.0)
                # preload the exp table set off the attention critical path
                warm = stat.tile([8, 1], F32, tag="warm")
                nc.scalar.activation(out=warm, in_=eps_sb, func=AF.Exp, scale=1.0)
                rm = stat.tile([8, 2], F32, tag="rm")  # [rstd_g, mean_g]
                nc.vector.reciprocal(rm[:, 0:1], std_g)
                nc.vector.tensor_copy(rm[:, 1:2], gstat[:, 0:1])
                # broadcast to channels: [rstd_c, mean_c] = H_t.T @ rm
                ab = []
                for t in range(2):
                    psb = psmm.tile([128, 2], F32, tag="mm")
                    nc.tensor.matmul(psb, hind[t], rm, start=True, stop=True)
                    abt = stat.tile([128, 2], F32, tag=f"ab{t}")  # [a_c, b_c]
                    nc.vector.tensor_mul(abt[:, 0:1], psb[:, 0:1], g_sb[t])
                    nc.vector.tensor_mul(abt[:, 1:2], psb[:, 1:2], abt[:, 0:1])
                    nc.vector.tensor_sub(abt[:, 1:2], be_sb[t], abt[:, 1:2])
                    ab.append(abt)

                # ---------- fold GN into weights: no x_norm pass ----------
                # K/Q/V2 consume RAW x; W' = W * a (per c_in), biases get W@b.
                # Bias matmuls (plain fp32, N=1-2) use the UNSCALED weights;
                # the in-place scales below are WAR-ordered after them.
                wTs = []
                w2ts = []
                for t in range(2):
                    wt2 = wpool.tile([128, 512], BF, tag=f"wTs{t}", name=f"wTs{t}")
                    nc.vector.tensor_scalar_mul(wt2, wT[t], ab[t][:, 0:1])
                    wTs.append(wt2)
                    w22 = wpool.tile([128, 256], BF, tag=f"w2ts{t}", name=f"w2ts{t}")
                    nc.vector.tensor_scalar_mul(w22, w2t[t], ab[t][:, 0:1])
                    w2ts.append(w22)
                ps_qb = psmm.tile([128, 4], F32, tag="mm", name="ps_qb")
                for m in range(4):
                    nc.tensor.matmul(
                        ps_qb[:, m : m + 1],
                        wT[0][:, m * 128 : (m + 1) * 128],
                        ab[0][:, 1:2],
                        start=True, stop=False,
                    )
                    nc.tensor.matmul(
                        ps_qb[:, m : m + 1],
                        wT[1][:, m * 128 : (m + 1) * 128],
                        ab[1][:, 1:2],
                        start=False, stop=True,
                    )
                qb_eff = stat.tile([128, 4], F32, tag="qbeff")
                nc.vector.tensor_add(qb_eff, ps_qb, qb_sb[:, 0:4])
                ps_ob2 = psmm.tile([128, 2], F32, tag="mm", name="ps_ob2")
                for m2 in range(2):
                    nc.tensor.matmul(
                        ps_ob2[:, m2 : m2 + 1],
                        w2t[0][:, m2 * 128 : (m2 + 1) * 128],
                        ab[0][:, 1:2],
                        start=True, stop=False,
                    )
                    nc.tensor.matmul(
                        ps_ob2[:, m2 : m2 + 1],
                        w2t[1][:, m2 * 128 : (m2 + 1) * 128],
                        ab[1][:, 1:2],
                        start=False, stop=True,
                    )
                ob_f = stat.tile([128, 2], F32, tag="obf")
                nc.vector.tensor_add(ob_f, ps_ob2, ob_eff)

                # residual prep from raw xq bits
                y_sb = []
                for t in range(2):
                    yt = ypool.tile([128, NQ], F32, tag="y", name=f"y{t}")
                    nc.vector.tensor_scalar_add(
                        yt, xq_sb[t][:, :], ob_f[:, t : t + 1]
                    )
                    y_sb.append(yt)
                xn = x_sb
                xqn = xq_sb

                # ---------- qkv projections ----------
                # Q and K are written as fp8e4 in DoubleRow layout
                # [128, 2, n] (c-halves packed in the middle dim) so the
                # scores matmul runs one fp8 DR matmul per (kt, qc).
                q8 = qpool.tile([128, 2, NQ], F8, tag="q", name="q8")
                k8 = kpool.tile([128, 2, HW], F8, tag="k", name="k8")
                nch = 0
                for m in (2, 3, 0, 1):
                    dst = q8 if m < 2 else k8
                    src = xqn if m < 2 else xn
                    nj = (NQ if m < 2 else HW) // 1024
                    for j in range(nj):
                        ps = psmm.tile([128, 1024], F32, tag="mm")
                        for i in range(2):
                            sl = slice((2 * j + i) * 512, (2 * j + i + 1) * 512)
                            nc.tensor.matmul(
                                ps[:, i * 512 : (i + 1) * 512],
                                wTs[0][:, m * 128 : (m + 1) * 128],
                                src[0][:, sl],
                                start=True,
                                stop=False,
                            )
                            nc.tensor.matmul(
                                ps[:, i * 512 : (i + 1) * 512],
                                wTs[1][:, m * 128 : (m + 1) * 128],
                                src[1][:, sl],
                                start=False,
                                stop=True,
                            )
                        dslice = dst[:, m % 2, j * 1024 : (j + 1) * 1024]
                        if nch % 2 == 0:
                            nc.vector.tensor_scalar_add(dslice, ps, qb_eff[:, m : m + 1])
                        else:
                            nc.scalar.activation(
                                out=dslice, in_=ps, func=AF.Identity,
                                bias=qb_eff[:, m : m + 1], scale=1.0,
                            )
                        nch += 1

                # ---------- V2^T = xn^T @ W2^T, fp8 [k-tile, c] ----------
                # v28[p, nt, c] = V2^T[nt*128+p, c]; PV runs transposed
                # (stationary = V2T k-pair slices, moving = P^T) so the
                # stationary is reused and fp8 DoubleRow halves the stream.
                v28 = vpool.tile([128, 32, 256], F8, tag="v2", name="v28")
                for ntq in range(8):
                    ps = psmm.tile([128, 1024], F32, tag="mm")
                    for i in range(4):
                        nt = 4 * ntq + i
                        nc.tensor.matmul(
                            ps[:, i * 256 : (i + 1) * 256],
                            xn[0][:, nt * 128 : (nt + 1) * 128], w2ts[0],
                            start=True, stop=False,
                        )
                        nc.tensor.matmul(
                            ps[:, i * 256 : (i + 1) * 256],
                            xn[1][:, nt * 128 : (nt + 1) * 128], w2ts[1],
                            start=False, stop=True,
                        )
                    dst = v28[:, 4 * ntq : 4 * ntq + 4, :]
                    if ntq % 2 == 0:
                        nc.vector.tensor_copy(dst, ps)
                    else:
                        nc.scalar.copy(dst, ps)
                # fp8 DR stationary needs middle-dim step % 16 == 0 and
                # memset can't write fp8: build the ones stationary as
                # [128, 2, 16] via an ACT copy from an f32 ones tile.
                ones_f = singles.tile([128, 32], F32, tag="onesf")
                nc.vector.memset(ones_f, 1.0)
                ones8 = singles.tile([128, 2, 16], F8, tag="ones8")
                nc.scalar.copy(ones8, ones_f)
                ones1f = singles.tile([1, 128], F32, tag="ones1f")
                nc.vector.memset(ones1f, 1.0)
                ones1 = singles.tile([1, 128], FR, tag="ones1")
                nc.vector.tensor_copy(ones1, ones1f)

                if debug:
                    for t in range(2):
                        nc.sync.dma_start(
                            out=d_xn[t * 128 : (t + 1) * 128, :],
                            in_=xn[t][:, :],
                        )
                        nc.sync.dma_start(
                            out=d_ab[t * 128 : (t + 1) * 128, :], in_=ab[t]
                        )

                # ---------- attention ----------
                # Per 512-query chunk (4 chunks): the two S^T k-tiles of a
                # jp-pair land in the two banks of ONE [128,2,512] PSUM tile
                # (ring 3 = 6 banks; matmul outputs may not cross a bank, but
                # exp reads may), so ONE exp per pair [128,2,512]->pT8
                # halves the exp instruction count. PV trails by 1 pair and
                # accumulates O'^T [128,512] x2 c-halves (2 banks). All 16
                # pT8 tiles stay live; the softmax denominator l is a
                # SEPARATE pass of 16 back-to-back DR matmuls with one ones
                # stationary load, into a ring slot vacated by S^T. 1/l is
                # PE-broadcast and applied on DVE (ch0) + Pool (ch1).
                pctx.close()
                actx = contextlib.ExitStack()
                psattn = actx.enter_context(
                    tc.tile_pool(name="psattn", bufs=2, space="PSUM")
                )
                pspo = actx.enter_context(
                    tc.tile_pool(name="pspo", bufs=4, space="PSUM")
                )
                NQB = 512
                for qg in range(NQ // (2 * NQB)):
                    # two query chunks (a=0, b=1) share every K/V stationary
                    # load: ldw(kt) then S-mm for both chunks back to back.
                    qsl = [
                        slice((2 * qg + h) * NQB, (2 * qg + h + 1) * NQB)
                        for h in range(2)
                    ]
                    poT = [
                        [
                            pspo.tile([128, NQB], F32, tag="o", name=f"poT{h}{ch}")
                            for ch in range(2)
                        ]
                        for h in range(2)
                    ]

                    def pv_stage(jp, pT8h):
                        for ch in range(2):
                            for h in range(2):
                                nc.tensor.matmul(
                                    poT[h][ch],
                                    v28[:, 2 * jp : 2 * jp + 2, ch * 128 : (ch + 1) * 128],
                                    pT8h[h],
                                    start=(jp == 0), stop=(jp == 15),
                                    perf_mode=mybir.MatmulPerfMode.DoubleRow,
                                    skip_group_check=True,
                                )

                    pv_pend = []
                    pT8s = [[], []]
                    for jp in range(16):
                        pT8h = [
                            ppool.tile([128, 2, NQB], F8, tag="p", name=f"pT8_{jp}_{h}")
                            for h in range(2)
                        ]
                        for h in range(2):
                            pT8s[h].append(pT8h[h])
                        psph = [
                            psattn.tile([128, 2, NQB], F32, tag="pair", name=f"psp_{jp}_{h}")
                            for h in range(2)
                        ]
                        for i in range(2):
                            kt = 2 * jp + i
                            for h in range(2):
                                nc.tensor.matmul(
                                    psph[h][:, i, :],
                                    k8[:, :, kt * 128 : (kt + 1) * 128],
                                    q8[:, :, qsl[h]],
                                    start=True, stop=True,
                                    perf_mode=mybir.MatmulPerfMode.DoubleRow,
                                )
                        for h in range(2):
                            nc.scalar.activation(
                                out=pT8h[h], in_=psph[h], func=AF.Exp,
                                scale=1.0 / 16.0, bias=ebias_sb,
                            )
                        pv_pend.append((jp, pT8h))
                        if len(pv_pend) > 1:
                            pv_stage(*pv_pend.pop(0))
                    for it in pv_pend:
                        pv_stage(*it)
                    # denominator pass: one stationary load, 32 chained mms
                    pol = [
                        psattn.tile([2, NQB], F32, tag="pair", name=f"pol{h}")
                        for h in range(2)
                    ]
                    for jp in range(16):
                        for h in range(2):
                            nc.tensor.matmul(
                                pol[h],
                                ones8[:, :, 0:2],
                                pT8s[h][jp],
                                start=(jp == 0), stop=(jp == 15),
                                perf_mode=mybir.MatmulPerfMode.DoubleRow,
                                skip_group_check=True,
                            )
                    for h in range(2):
                        # rl = 1/l broadcast to all 128 partitions via PE
                        rlv = small.tile([1, NQB], FR, tag="rlv")
                        with nc.allow_low_precision(reason="rl bcast via f32r matmul"):
                            nc.vector.reciprocal(rlv, pol[h][0:1, :])
                        rlb = psattn.tile([128, NQB], F32, tag="pair", name=f"rlb{h}")
                        nc.tensor.matmul(rlb, ones1, rlv, start=True, stop=True)
                        # engines may read only one PSUM operand per instruction
                        rlb_sb = opool.tile([128, NQB], F32, tag="rlbs", name=f"rlb_sb{h}")
                        nc.vector.tensor_copy(rlb_sb, rlb)
                        tmps = []
                        for t in range(2):
                            tmp = opool.tile([128, NQB], F32, tag="tmp", name=f"tmp{h}{t}")
                            nc.vector.tensor_tensor(tmp, poT[h][t], rlb_sb, ALU.mult)
                            tmps.append(tmp)
                        for t in range(2):
                            ys = y_sb[t][:, qsl[h]]
                            nc.gpsimd.tensor_tensor(ys, tmps[t], ys, ALU.add)
                        for t in range(2):
                            nc.sync.dma_start(
                                out=y[t * 128 : (t + 1) * 128, qsl[h]],
                                in_=y_sb[t][:, qsl[h]],
                            )
                actx.close()

            consts = setup()
            if hw_loop and loop_reps > 1:
                with tc.For_i(0, loop_reps) as _i:
                    body(0, consts)
            else:
                for rep in range(loop_reps):
                    body(rep, consts)

    return nc


def _get_runner(loop_reps=1):
    key = ("runner", loop_reps)
    if key not in _CACHE:
        nc = _build_nc(loop_reps)
        _CACHE[key] = nc
    return _CACHE[key]


K_USE_FP8 = USE_FP8_PV


def make_extra_inputs():
    gind = np.zeros((128, 16), dtype=np.float32)
    hind = np.zeros((8, 256), dtype=np.float32)
    for t in range(2):
        for p in range(128):
            g = (t * 128 + p) // CPG
            gind[p, t * 8 + g] = 1.0
            hind[g, t * 128 + p] = 1.0
    return {"gind_in": gind, "hind_in": hind}


def make_weight_inputs(qkv_w, out_w, qkv_b, out_b):
    # host-side static folds: W2 = out_w @ Wv; out_b' = out_b + out_w @ bv
    qkv_w = np.asarray(qkv_w, dtype=np.float32)
    out_w = np.asarray(out_w, dtype=np.float32)
    w2 = out_w @ qkv_w[512:768]
    ob = np.asarray(out_b, dtype=np.float32) + out_w @ np.asarray(
        qkv_b, dtype=np.float32
    )[512:768]
    return {
        "wqkT_in": np.ascontiguousarray(qkv_w[0:512].T),
        "w2T_in": np.ascontiguousarray(w2.T),
        "out_b": ob,
    }


def kernel(x, gn_gamma, gn_beta, qkv_w, qkv_b, out_w, out_b):
    from concourse.bass_utils import run_bass_kernel_spmd

    x = np.asarray(x, dtype=np.float32)
    gn_gamma = np.asarray(gn_gamma, dtype=np.float32)
    gn_beta = np.asarray(gn_beta, dtype=np.float32)
    qkv_w = np.asarray(qkv_w, dtype=np.float32)
    qkv_b = np.asarray(qkv_b, dtype=np.float32)
    out_w = np.asarray(out_w, dtype=np.float32)
    out_b = np.asarray(out_b, dtype=np.float32)

    b, c, h, w = x.shape
    assert (b, c, h * w) == (B, C, HW)
    xf = x.reshape(b, c, HW)

    nc = _get_runner()
    in_maps = []
    for j in range(N_CORES):
        bi, qh = j // 2, j % 2
        if qh == 0:
            xbj = np.ascontiguousarray(xf[bi])
        else:
            xbj = np.concatenate([xf[bi][:, NQ:], xf[bi][:, :NQ]], axis=1)
        xbj = xbj.astype(_bf16())
        in_maps.append(
            {
                "xb": xbj,
                "qkv_b": qkv_b,
                "out_b": out_b,
                "gn_gamma": gn_gamma,
                "gn_beta": gn_beta,
            }
        )
    extras = make_extra_inputs()
    extras.update(make_weight_inputs(qkv_w, out_w, qkv_b, out_b))
    for m in in_maps:
        m.update(extras)
    res = run_bass_kernel_spmd(nc, in_maps, core_ids=list(range(N_CORES)))
    out = np.empty((B, C, HW), dtype=np.float32)
    for j in range(N_CORES):
        bi, qh = j // 2, j % 2
        out[bi][:, qh * NQ : (qh + 1) * NQ] = res.results[j]["y"]
    return out.reshape(b, c, h, w)

